# revision 47
# baseline (speedup 1.0000x reference)
"""Trainium2 Bass kernel for nn_Attention (dense_transformer).

Reference computation (per batch b):
    scores  = Q @ M^T                  # (T,S), contraction over H
    attn    = softmax(scores, axis=S)  # mask is all-False (fill=zeros) -> no-op
    context = attn @ M                 # (T,H)
    out     = tanh([context, Q] @ W^T + b)
Returns (out, attn) like the reference.

Distribution: data-parallel over B=16 across 8 cores (2 batches/core).
Compute dtype: fp16 operands with fp32 PSUM accumulation (1 cyc/row on PE;
fp32 matmul would be 4 cyc/row).  Softmax + outputs in fp32.

Layouts (all built on-chip via DMA-transpose / natural DMA from fp16 DRAM):
    QT  = Q^T  [H,T]   (stationary for scores, lhsT for out-matmul)
    MT  = M^T  [H,S]   (moving for scores)
    Mn  = M    [S,H]   (stationary for context^T)
    attnT [S,T]        (moving for context^T, via SBUF->SBUF DMA-transpose)
    context is computed transposed (ctxT [H,T]) so the final matmul needs
    no extra transposes: out = [ctxT; QT]^T @ W^T (+ ones-row trick for bias).
"""

import numpy as np
from contextlib import ExitStack

T, B, H, S = 1024, 16, 1024, 2048
NCORES = 8
BL = B // NCORES  # batches per core


def build_nc(t=T, s=S, h=H, bl=BL, INTERLEAVE=False):
    import concourse.bass as bass  # noqa: F401
    import concourse.mybir as mybir
    import concourse.tile as tile
    from concourse import bacc

    dt = mybir.dt
    f16, f32 = dt.float16, dt.float32
    AF = mybir.ActivationFunctionType
    AX = mybir.AxisListType

    TT = t // 128       # t tiles
    TC = t // 512       # t chunks (4 t-tiles each)
    SC = s // 512       # s chunks for scores matmul
    HT = h // 128       # h tiles (also k-steps over H)
    ST = s // 128       # s tiles (k-steps over S)
    DT = 2 * h // 128   # k-steps over 2H for the final matmul
    HC = h // 512       # h' chunks for the final matmul

    nc = bacc.Bacc()
    m16 = nc.declare_dram_parameter("m16", [s, bl, h], f16, isOutput=False)
    qT16 = nc.declare_dram_parameter("qT16", [bl, h, t], f16, isOutput=False)
    mT16 = nc.declare_dram_parameter("mT16", [bl, h, s], f16, isOutput=False)
    qlT16 = nc.declare_dram_parameter("qlT16", [bl, h, t], f16, isOutput=False)
    mlT16 = nc.declare_dram_parameter("mlT16", [bl, h, s], f16, isOutput=False)
    wt16 = nc.declare_dram_parameter("wt16", [2 * h, h], f16, isOutput=False)
    b16 = nc.declare_dram_parameter("b16", [1, h], f16, isOutput=False)
    out_e = nc.declare_dram_parameter("out", [t, bl, h], f32, isOutput=True)
    attn_e = nc.declare_dram_parameter("attn", [bl, t, s], f32, isOutput=True)

    with ExitStack() as ctx:
        tc = ctx.enter_context(tile.TileContext(nc))
        consts = ctx.enter_context(tc.tile_pool(name="consts", bufs=1))
        lay = ctx.enter_context(tc.tile_pool(name="lay", bufs=1))
        work1 = ctx.enter_context(tc.tile_pool(name="work1", bufs=1))
        ctxp = ctx.enter_context(tc.tile_pool(name="ctxp", bufs=2))
        e16p = ctx.enter_context(tc.tile_pool(name="e16p", bufs=2))
        stats = ctx.enter_context(tc.tile_pool(name="stats", bufs=4))
        # one unified psum pool: 4 rotating slots of [128,1024] (2 banks each)
        psq = ctx.enter_context(tc.tile_pool(name="psq", bufs=4, space="PSUM"))
        dramp = ctx.enter_context(tc.tile_pool(name="dramp", bufs=2,
                                               space="DRAM"))
        hwdge = (nc.sync, nc.scalar)  # spread engine-serial DMA transposes

        # W^T (2H,H) as DT tiles of [128, H]; loaded after the first batch's
        # hi layouts (WT is only needed late, by the out-matmuls)
        WT = consts.tile([128, DT, h], f16, tag="WT")
        bsb = consts.tile([1, h], f16, tag="bsb")
        nc.sync.dma_start(out=bsb[:], in_=b16[:])
        ones = consts.tile([1, 128], f16, tag="ones")
        nc.vector.memset(ones[:], 1.0)

        def emit_out(QTb, ctxTb, bb, tch, i):
            # out = tanh([ctx, Q] @ W^T + b) for t-tile i of chunk tch
            ti = tch * 4 + i
            osb = work1.tile([128, h], f32, tag="osb")
            po = psq.tile([128, 1024], f32, tag="psq", name=f"po_{bb}_{ti}")
            for hp in range(HC):
                pos = po[:, 512 * hp:512 * (hp + 1)]
                for kd in range(HT):
                    nc.tensor.matmul(
                        pos,
                        lhsT=ctxTb[:, kd, 128 * i:128 * (i + 1)],
                        rhs=WT[:, kd, 512 * hp:512 * (hp + 1)],
                        start=(kd == 0), stop=False)
                for kd in range(HT):
                    nc.tensor.matmul(
                        pos,
                        lhsT=QTb[kd][:, 128 * ti:128 * (ti + 1)],
                        rhs=WT[:, HT + kd, 512 * hp:512 * (hp + 1)],
                        start=False, stop=False)
                nc.tensor.matmul(
                    pos, lhsT=ones,
                    rhs=bsb[:, 512 * hp:512 * (hp + 1)],
                    start=False, stop=True)
                nc.scalar.activation(
                    out=osb[:, 512 * hp:512 * (hp + 1)], in_=pos,
                    func=AF.Tanh)
            nc.gpsimd.dma_start(
                out=out_e[128 * ti:128 * (ti + 1), bb, :], in_=osb)

        def emit_ctx_pair(Mnb, attnTb, ctxTb, bb, tch, hq):
            # one pair of context^T h-tiles into a rotating psum slot
            pc = psq.tile([128, 1024], f32, tag="psq",
                          name=f"pc_{bb}_{tch}_{hq}")
            for j in range(ST):
                for u in range(2):
                    hh = 2 * hq + u
                    nc.tensor.matmul(
                        pc[:, 512 * u:512 * (u + 1)],
                        lhsT=Mnb[j][:, 128 * hh:128 * (hh + 1)],
                        rhs=attnTb[j // (ST // 4)][:, j % (ST // 4), :],
                        start=(j == 0), stop=(j == ST - 1))
            for u in range(2):
                nc.scalar.copy(out=ctxTb[:, 2 * hq + u, :],
                               in_=pc[:, 512 * u:512 * (u + 1)])

        # ctx work of chunk c is deferred and drained (1 item per t-tile)
        # during chunk c+1's scores; out work of chunk c drains right after
        # chunk c+1's transpose-reads are issued, so PE stays busy while the
        # reads stream on the sync engine
        pcwork, powork = [], []
        for b in range(bl):
            # per-batch input layouts (hi + lo residual for exact-ish scores),
            # as per-h-chunk tiles so consumers start as soon as each chunk
            # lands.  hi layouts first: scores pass 1 only needs QT/MT.
            QT = [lay.tile([128, t], f16, tag=f"QT{hc}", name=f"QT{hc}_{b}") for hc in range(HT)]
            MT = [lay.tile([128, s], f16, tag=f"MT{hc}", name=f"MT{hc}_{b}") for hc in range(HT)]
            QTl = [lay.tile([128, t], f16, tag=f"QTl{hc}", name=f"QTl{hc}_{b}") for hc in range(HT)]
            MTl = [lay.tile([128, s], f16, tag=f"MTl{hc}", name=f"MTl{hc}_{b}") for hc in range(HT)]
            Mn = [lay.tile([128, h], f16, tag=f"Mn{st}", name=f"Mn{st}_{b}") for st in range(ST)]
            for hc in range(HT):
                hsl = slice(128 * hc, 128 * (hc + 1))
                nc.sync.dma_start(out=QT[hc][:], in_=qT16[b, hsl, :])
                nc.sync.dma_start(out=MT[hc][:], in_=mT16[b, hsl, :])
            for hc in range(HT):
                hsl = slice(128 * hc, 128 * (hc + 1))
                nc.sync.dma_start(out=QTl[hc][:], in_=qlT16[b, hsl, :])
                nc.sync.dma_start(out=MTl[hc][:], in_=mlT16[b, hsl, :])
            for st in range(ST):
                nc.sync.dma_start(
                    out=Mn[st][:], in_=m16[128 * st:128 * (st + 1), b, :])

            for tch in range(TC):
                attnT = [work1.tile([128, ST // 4, 512], f16,
                                    tag=f"attnT{k}", name=f"attnT{k}_{b}_{tch}")
                         for k in range(4)]
                e16d = dramp.tile([512, s], f16, tag="e16d",
                                  name=f"e16d_{b}_{tch}")
                for i in range(4):
                    ti = tch * 4 + i
                    tsl = slice(128 * ti, 128 * (ti + 1))
                    # scores for t-tile ti, in two s-halves, each a rotating
                    # psum slot: next tile's matmuls never wait on this
                    # tile's softmax drain.
                    psh = []
                    mx = stats.tile([128, 2], f32, tag="mx")
                    pairs = ((QT, MT), (QTl, MT), (QT, MTl))
                    sh = s // 2
                    for hf in range(2):
                        ph = psq.tile([128, sh], mybir.dt.float32,
                                      tag="psq", name=f"ps_{b}_{ti}_{hf}")
                        psh.append(ph)
                        for pi, (L, R) in enumerate(pairs):
                            for kh in range(HT):
                                for s2 in range(sh // 512):
                                    nc.tensor.matmul(
                                        ph[:, 512 * s2:512 * (s2 + 1)],
                                        lhsT=L[kh][:, tsl],
                                        rhs=R[kh][:, sh * hf + 512 * s2:
                                                   sh * hf + 512 * (s2 + 1)],
                                        start=(pi == 0 and kh == 0),
                                        stop=(pi == 2 and kh == HT - 1))
                        nc.vector.reduce_max(out=mx[:, hf:hf + 1], in_=ph[:],
                                             axis=AX.X)
                    negmax = stats.tile([128, 1], f32, tag="negmax")
                    nc.vector.reduce_max(out=negmax, in_=mx[:], axis=AX.X,
                                         negate=True)
                    e16 = e16p.tile([128, s], f16, tag="e16")
                    ssum = stats.tile([128, 2], f32, tag="ssum")
                    for hf in range(2):
                        nc.scalar.activation(
                            out=e16[:, sh * hf:sh * (hf + 1)],
                            in_=psh[hf][:], func=AF.Exp, bias=negmax,
                            scale=1.0, accum_out=ssum[:, hf:hf + 1])
                    rsum = stats.tile([128, 1], f32, tag="rsum")
                    nc.vector.reduce_sum(out=rsum, in_=ssum[:], axis=AX.X)
                    recip = stats.tile([128, 1], f32, tag="recip")
                    nc.vector.reciprocal(recip, rsum)
                    # normalize in place; attn output via casting SWDGE DMA,
                    # fp16 copy to DRAM scratch for the big transpose-reads
                    nc.vector.tensor_scalar_mul(e16, e16, recip)
                    nc.sync.dma_start(
                        out=e16d[128 * i:128 * (i + 1), :], in_=e16)
                    nc.gpsimd.dma_start(
                        out=attn_e[b, 128 * ti:128 * (ti + 1), :], in_=e16)
                    for _ in range(2):
                        if pcwork:
                            pcwork.pop(0)()
                    if b == 0 and tch == 0:
                        # spread weight loads; first needed by the outs of
                        # chunk 0, drained a whole chunk later
                        for kt in range(i * DT // 4, (i + 1) * DT // 4):
                            nc.sync.dma_start(
                                out=WT[:, kt, :],
                                in_=wt16[128 * kt:128 * (kt + 1), :])
                # attn^T for the whole chunk: ST transpose-reads of
                # [512,128] from the DRAM scratch, split over both HWDGE
                # engines (engine-serial, ~1.4us each)
                for j in range(ST):
                    nc.scalar.dma_start_transpose(
                        attnT[j // (ST // 4)][:, j % (ST // 4), :],
                        e16d[:, 128 * j:128 * (j + 1)])
                while powork:
                    powork.pop(0)()

                ctxT = ctxp.tile([128, HT, 512], f16, tag="ctxT",
                                 name=f"ctxT_{b}_{tch}")
                for hq in range(HT // 2):
                    pcwork.append(
                        lambda Mnb=Mn, aT=attnT, cT=ctxT, bb=b, tc_=tch,
                        hq_=hq: emit_ctx_pair(Mnb, aT, cT, bb, tc_, hq_))
                for i in range(4):
                    powork.append(
                        lambda QTb=QT, cT=ctxT, bb=b, tc_=tch, i_=i:
                        emit_out(QTb, cT, bb, tc_, i_))
            # flush before the next batch's layouts overwrite QT/Mn (their
            # WAR would otherwise put deferred work behind the next batch's
            # scores in the PE stream -> deadlock)
            while pcwork:
                pcwork.pop(0)()
            while powork:
                powork.pop(0)()
    nc.finalize()
    return nc


def kernel(output, memory, attn_mask, W, b):
    from concourse.bass_utils import run_bass_kernel_spmd

    qf = np.asarray(output, dtype=np.float32).reshape(T, B, H)
    mf = np.asarray(memory, dtype=np.float32).reshape(S, B, H)
    q16 = qf.astype(np.float16)
    m16 = mf.astype(np.float16)
    q16l = (qf - q16.astype(np.float32)).astype(np.float16)
    m16l = (mf - m16.astype(np.float32)).astype(np.float16)
    # pre-transposed layouts (B, H, T/S): the kernel's matmul operand layout
    qT16 = np.ascontiguousarray(q16.transpose(1, 2, 0))
    mT16 = np.ascontiguousarray(m16.transpose(1, 2, 0))
    qlT16 = np.ascontiguousarray(q16l.transpose(1, 2, 0))
    mlT16 = np.ascontiguousarray(m16l.transpose(1, 2, 0))
    wt16 = np.ascontiguousarray(np.asarray(W, dtype=np.float32).T,
                                dtype=np.float16)          # (2H, H)
    b16 = np.asarray(b, dtype=np.float16).reshape(1, H)

    nc = build_nc()
    in_maps = []
    for c in range(NCORES):
        lo, hi = c * BL, (c + 1) * BL
        in_maps.append({
            "m16": np.ascontiguousarray(m16[:, lo:hi, :]),
            "qT16": np.ascontiguousarray(qT16[lo:hi]),
            "mT16": np.ascontiguousarray(mT16[lo:hi]),
            "qlT16": np.ascontiguousarray(qlT16[lo:hi]),
            "mlT16": np.ascontiguousarray(mlT16[lo:hi]),
            "wt16": wt16,
            "b16": b16,
        })
    res = run_bass_kernel_spmd(nc, in_maps, core_ids=list(range(NCORES)))
    out_full = np.concatenate([r["out"] for r in res.results], axis=1)
    attn_full = np.concatenate([r["attn"] for r in res.results], axis=0)
    return out_full, attn_full


# revision 48
# speedup vs baseline: 1.0867x; 1.0867x over previous
"""Trainium2 Bass kernel for nn_Attention (dense_transformer).

Reference computation (per batch b):
    scores  = Q @ M^T                  # (T,S), contraction over H
    attn    = softmax(scores, axis=S)  # mask is all-False (fill=zeros) -> no-op
    context = attn @ M                 # (T,H)
    out     = tanh([context, Q] @ W^T + b)
Returns (out, attn) like the reference.

Distribution: data-parallel over B=16 across 8 cores (2 batches/core).
Compute dtype: fp16 operands with fp32 PSUM accumulation (1 cyc/row on PE;
fp32 matmul would be 4 cyc/row).  Softmax + outputs in fp32.

Layouts (all built on-chip via DMA-transpose / natural DMA from fp16 DRAM):
    QT  = Q^T  [H,T]   (stationary for scores, lhsT for out-matmul)
    MT  = M^T  [H,S]   (moving for scores)
    Mn  = M    [S,H]   (stationary for context^T)
    attnT [S,T]        (moving for context^T, via SBUF->SBUF DMA-transpose)
    context is computed transposed (ctxT [H,T]) so the final matmul needs
    no extra transposes: out = [ctxT; QT]^T @ W^T (+ ones-row trick for bias).
"""

import numpy as np
from contextlib import ExitStack

T, B, H, S = 1024, 16, 1024, 2048
NCORES = 8
BL = B // NCORES  # batches per core


def build_nc(t=T, s=S, h=H, bl=BL, INTERLEAVE=False):
    import concourse.bass as bass  # noqa: F401
    import concourse.mybir as mybir
    import concourse.tile as tile
    from concourse import bacc

    dt = mybir.dt
    f16, f32 = dt.float16, dt.float32
    AF = mybir.ActivationFunctionType
    AX = mybir.AxisListType

    TT = t // 128       # t tiles
    TC = t // 512       # t chunks (4 t-tiles each)
    SC = s // 512       # s chunks for scores matmul
    HT = h // 128       # h tiles (also k-steps over H)
    ST = s // 128       # s tiles (k-steps over S)
    DT = 2 * h // 128   # k-steps over 2H for the final matmul
    HC = h // 512       # h' chunks for the final matmul

    nc = bacc.Bacc()
    m16 = nc.declare_dram_parameter("m16", [s, bl, h], f16, isOutput=False)
    qT16 = nc.declare_dram_parameter("qT16", [bl, h, t], f16, isOutput=False)
    mT16 = nc.declare_dram_parameter("mT16", [bl, h, s], f16, isOutput=False)
    qlT16 = nc.declare_dram_parameter("qlT16", [bl, h, t], f16, isOutput=False)
    mlT16 = nc.declare_dram_parameter("mlT16", [bl, h, s], f16, isOutput=False)
    wt16 = nc.declare_dram_parameter("wt16", [2 * h, h], f16, isOutput=False)
    b16 = nc.declare_dram_parameter("b16", [1, h], f16, isOutput=False)
    out_e = nc.declare_dram_parameter("out", [t, bl, h], f32, isOutput=True)
    attn_e = nc.declare_dram_parameter("attn", [bl, t, s], f32, isOutput=True)

    with ExitStack() as ctx:
        tc = ctx.enter_context(tile.TileContext(nc))
        consts = ctx.enter_context(tc.tile_pool(name="consts", bufs=1))
        lay = ctx.enter_context(tc.tile_pool(name="lay", bufs=1))
        work1 = ctx.enter_context(tc.tile_pool(name="work1", bufs=1))
        ctxp = ctx.enter_context(tc.tile_pool(name="ctxp", bufs=2))
        e16p = ctx.enter_context(tc.tile_pool(name="e16p", bufs=2))
        stats = ctx.enter_context(tc.tile_pool(name="stats", bufs=4))
        # one unified psum pool: 4 rotating slots of [128,1024] (2 banks each)
        psq = ctx.enter_context(tc.tile_pool(name="psq", bufs=4, space="PSUM"))
        dramp = ctx.enter_context(tc.tile_pool(name="dramp", bufs=2,
                                               space="DRAM"))
        hwdge = (nc.sync, nc.scalar)  # spread engine-serial DMA transposes

        # W^T (2H,H) as DT tiles of [128, H]; loaded after the first batch's
        # hi layouts (WT is only needed late, by the out-matmuls)
        WT = consts.tile([128, DT, h], f16, tag="WT")
        bsb = consts.tile([1, h], f16, tag="bsb")
        nc.sync.dma_start(out=bsb[:], in_=b16[:])
        ones = consts.tile([1, 128], f16, tag="ones")
        nc.vector.memset(ones[:], 1.0)

        def emit_out(QTb, ctxTb, bb, tch, i):
            # out = tanh([ctx, Q] @ W^T + b) for t-tile i of chunk tch
            ti = tch * 4 + i
            osb = work1.tile([128, h], f32, tag="osb")
            po = psq.tile([128, 1024], f32, tag="psq", name=f"po_{bb}_{ti}")
            for hp in range(HC):
                pos = po[:, 512 * hp:512 * (hp + 1)]
                for kd in range(HT):
                    nc.tensor.matmul(
                        pos,
                        lhsT=ctxTb[:, kd, 128 * i:128 * (i + 1)],
                        rhs=WT[:, kd, 512 * hp:512 * (hp + 1)],
                        start=(kd == 0), stop=False)
                for kd in range(HT):
                    nc.tensor.matmul(
                        pos,
                        lhsT=QTb[kd][:, 128 * ti:128 * (ti + 1)],
                        rhs=WT[:, HT + kd, 512 * hp:512 * (hp + 1)],
                        start=False, stop=False)
                nc.tensor.matmul(
                    pos, lhsT=ones,
                    rhs=bsb[:, 512 * hp:512 * (hp + 1)],
                    start=False, stop=True)
                nc.scalar.activation(
                    out=osb[:, 512 * hp:512 * (hp + 1)], in_=pos,
                    func=AF.Tanh)
            nc.gpsimd.dma_start(
                out=out_e[128 * ti:128 * (ti + 1), bb, :], in_=osb)

        def emit_ctx_pair(Mnb, attnTb, ctxTb, bb, tch, hq):
            # one pair of context^T h-tiles into a rotating psum slot
            pc = psq.tile([128, 1024], f32, tag="psq",
                          name=f"pc_{bb}_{tch}_{hq}")
            for j in range(ST):
                for u in range(2):
                    hh = 2 * hq + u
                    nc.tensor.matmul(
                        pc[:, 512 * u:512 * (u + 1)],
                        lhsT=Mnb[j][:, 128 * hh:128 * (hh + 1)],
                        rhs=attnTb[j // (ST // 4)][:, j % (ST // 4), :],
                        start=(j == 0), stop=(j == ST - 1))
            for u in range(2):
                nc.scalar.copy(out=ctxTb[:, 2 * hq + u, :],
                               in_=pc[:, 512 * u:512 * (u + 1)])

        # ctx work of chunk c is deferred and drained (1 item per t-tile)
        # during chunk c+1's scores; out work of chunk c drains right after
        # chunk c+1's transpose-reads are issued, so PE stays busy while the
        # reads stream on the sync engine
        pcwork, powork = [], []
        for b in range(bl):
            # per-batch input layouts (hi + lo residual for exact-ish scores),
            # as per-h-chunk tiles so consumers start as soon as each chunk
            # lands.  hi layouts first: scores pass 1 only needs QT/MT.
            QT = [lay.tile([128, t], f16, tag=f"QT{hc}", name=f"QT{hc}_{b}") for hc in range(HT)]
            MT = [lay.tile([128, s], f16, tag=f"MT{hc}", name=f"MT{hc}_{b}") for hc in range(HT)]
            QTl = [lay.tile([128, t], f16, tag=f"QTl{hc}", name=f"QTl{hc}_{b}") for hc in range(HT)]
            MTl = [lay.tile([128, s], f16, tag=f"MTl{hc}", name=f"MTl{hc}_{b}") for hc in range(HT)]
            Mn = [lay.tile([128, h], f16, tag=f"Mn{st}", name=f"Mn{st}_{b}") for st in range(ST)]
            for hc in range(HT):
                hsl = slice(128 * hc, 128 * (hc + 1))
                nc.sync.dma_start(out=QT[hc][:], in_=qT16[b, hsl, :])
                nc.sync.dma_start(out=MT[hc][:], in_=mT16[b, hsl, :])
            for hc in range(HT):
                hsl = slice(128 * hc, 128 * (hc + 1))
                nc.sync.dma_start(out=QTl[hc][:], in_=qlT16[b, hsl, :])
                nc.sync.dma_start(out=MTl[hc][:], in_=mlT16[b, hsl, :])
            for st in range(ST):
                nc.sync.dma_start(
                    out=Mn[st][:], in_=m16[128 * st:128 * (st + 1), b, :])

            for tch in range(TC):
                attnT = [work1.tile([128, ST // 4, 512], f16,
                                    tag=f"attnT{k}", name=f"attnT{k}_{b}_{tch}")
                         for k in range(4)]
                e16d = dramp.tile([512, s], f16, tag="e16d",
                                  name=f"e16d_{b}_{tch}")
                for i in range(4):
                    ti = tch * 4 + i
                    tsl = slice(128 * ti, 128 * (ti + 1))
                    # scores for t-tile ti, in two s-halves, each a rotating
                    # psum slot: next tile's matmuls never wait on this
                    # tile's softmax drain.
                    psh = []
                    mx = stats.tile([128, 2], f32, tag="mx")
                    pairs = ((QT, MT), (QTl, MT), (QT, MTl))
                    sh = s // 2
                    for hf in range(2):
                        ph = psq.tile([128, sh], mybir.dt.float32,
                                      tag="psq", name=f"ps_{b}_{ti}_{hf}")
                        psh.append(ph)
                        for pi, (L, R) in enumerate(pairs):
                            for kh in range(HT):
                                for s2 in range(sh // 512):
                                    nc.tensor.matmul(
                                        ph[:, 512 * s2:512 * (s2 + 1)],
                                        lhsT=L[kh][:, tsl],
                                        rhs=R[kh][:, sh * hf + 512 * s2:
                                                   sh * hf + 512 * (s2 + 1)],
                                        start=(pi == 0 and kh == 0),
                                        stop=(pi == 2 and kh == HT - 1))
                        nc.vector.reduce_max(out=mx[:, hf:hf + 1], in_=ph[:],
                                             axis=AX.X)
                    negmax = stats.tile([128, 1], f32, tag="negmax")
                    nc.vector.reduce_max(out=negmax, in_=mx[:], axis=AX.X,
                                         negate=True)
                    e16 = e16p.tile([128, s], f16, tag="e16")
                    ssum = stats.tile([128, 2], f32, tag="ssum")
                    for hf in range(2):
                        nc.scalar.activation(
                            out=e16[:, sh * hf:sh * (hf + 1)],
                            in_=psh[hf][:], func=AF.Exp, bias=negmax,
                            scale=1.0, accum_out=ssum[:, hf:hf + 1])
                    rsum = stats.tile([128, 1], f32, tag="rsum")
                    nc.vector.reduce_sum(out=rsum, in_=ssum[:], axis=AX.X)
                    recip = stats.tile([128, 1], f32, tag="recip")
                    nc.vector.reciprocal(recip, rsum)
                    # normalize in place; attn output via casting SWDGE DMA,
                    # fp16 copy to DRAM scratch for the big transpose-reads
                    nc.vector.tensor_scalar_mul(e16, e16, recip)
                    nc.sync.dma_start(
                        out=e16d[128 * i:128 * (i + 1), :], in_=e16)
                    nc.gpsimd.dma_start(
                        out=attn_e[b, 128 * ti:128 * (ti + 1), :], in_=e16)
                    for _ in range(2):
                        if pcwork:
                            pcwork.pop(0)()
                    if b == 0 and tch == 0:
                        # spread weight loads; first needed by the outs of
                        # chunk 0, drained a whole chunk later
                        for kt in range(i * DT // 4, (i + 1) * DT // 4):
                            nc.sync.dma_start(
                                out=WT[:, kt, :],
                                in_=wt16[128 * kt:128 * (kt + 1), :])
                # attn^T for the whole chunk: ST transpose-reads of
                # [512,128] from the DRAM scratch, split over both HWDGE
                # engines (engine-serial, ~1.4us each)
                for j in range(ST):
                    nc.sync.dma_start_transpose(
                        attnT[j // (ST // 4)][:, j % (ST // 4), :],
                        e16d[:, 128 * j:128 * (j + 1)])
                while powork:
                    powork.pop(0)()

                ctxT = ctxp.tile([128, HT, 512], f16, tag="ctxT",
                                 name=f"ctxT_{b}_{tch}")
                for hq in range(HT // 2):
                    pcwork.append(
                        lambda Mnb=Mn, aT=attnT, cT=ctxT, bb=b, tc_=tch,
                        hq_=hq: emit_ctx_pair(Mnb, aT, cT, bb, tc_, hq_))
                for i in range(4):
                    powork.append(
                        lambda QTb=QT, cT=ctxT, bb=b, tc_=tch, i_=i:
                        emit_out(QTb, cT, bb, tc_, i_))
            # flush before the next batch's layouts overwrite QT/Mn (their
            # WAR would otherwise put deferred work behind the next batch's
            # scores in the PE stream -> deadlock)
            while pcwork:
                pcwork.pop(0)()
            while powork:
                powork.pop(0)()
    nc.finalize()
    return nc


def kernel(output, memory, attn_mask, W, b):
    from concourse.bass_utils import run_bass_kernel_spmd

    qf = np.asarray(output, dtype=np.float32).reshape(T, B, H)
    mf = np.asarray(memory, dtype=np.float32).reshape(S, B, H)
    q16 = qf.astype(np.float16)
    m16 = mf.astype(np.float16)
    q16l = (qf - q16.astype(np.float32)).astype(np.float16)
    m16l = (mf - m16.astype(np.float32)).astype(np.float16)
    # pre-transposed layouts (B, H, T/S): the kernel's matmul operand layout
    qT16 = np.ascontiguousarray(q16.transpose(1, 2, 0))
    mT16 = np.ascontiguousarray(m16.transpose(1, 2, 0))
    qlT16 = np.ascontiguousarray(q16l.transpose(1, 2, 0))
    mlT16 = np.ascontiguousarray(m16l.transpose(1, 2, 0))
    wt16 = np.ascontiguousarray(np.asarray(W, dtype=np.float32).T,
                                dtype=np.float16)          # (2H, H)
    b16 = np.asarray(b, dtype=np.float16).reshape(1, H)

    nc = build_nc()
    in_maps = []
    for c in range(NCORES):
        lo, hi = c * BL, (c + 1) * BL
        in_maps.append({
            "m16": np.ascontiguousarray(m16[:, lo:hi, :]),
            "qT16": np.ascontiguousarray(qT16[lo:hi]),
            "mT16": np.ascontiguousarray(mT16[lo:hi]),
            "qlT16": np.ascontiguousarray(qlT16[lo:hi]),
            "mlT16": np.ascontiguousarray(mlT16[lo:hi]),
            "wt16": wt16,
            "b16": b16,
        })
    res = run_bass_kernel_spmd(nc, in_maps, core_ids=list(range(NCORES)))
    out_full = np.concatenate([r["out"] for r in res.results], axis=1)
    attn_full = np.concatenate([r["attn"] for r in res.results], axis=0)
    return out_full, attn_full


# revision 49
# speedup vs baseline: 1.1398x; 1.0489x over previous
"""Trainium2 Bass kernel for nn_Attention (dense_transformer).

Reference computation (per batch b):
    scores  = Q @ M^T                  # (T,S), contraction over H
    attn    = softmax(scores, axis=S)  # mask is all-False (fill=zeros) -> no-op
    context = attn @ M                 # (T,H)
    out     = tanh([context, Q] @ W^T + b)
Returns (out, attn) like the reference.

Distribution: data-parallel over B=16 across 8 cores (2 batches/core).
Compute dtype: fp16 operands with fp32 PSUM accumulation (1 cyc/row on PE;
fp32 matmul would be 4 cyc/row).  Softmax + outputs in fp32.

Layouts (all built on-chip via DMA-transpose / natural DMA from fp16 DRAM):
    QT  = Q^T  [H,T]   (stationary for scores, lhsT for out-matmul)
    MT  = M^T  [H,S]   (moving for scores)
    Mn  = M    [S,H]   (stationary for context^T)
    attnT [S,T]        (moving for context^T, via SBUF->SBUF DMA-transpose)
    context is computed transposed (ctxT [H,T]) so the final matmul needs
    no extra transposes: out = [ctxT; QT]^T @ W^T (+ ones-row trick for bias).
"""

import numpy as np
from contextlib import ExitStack

T, B, H, S = 1024, 16, 1024, 2048
NCORES = 8
BL = B // NCORES  # batches per core


def build_nc(t=T, s=S, h=H, bl=BL, INTERLEAVE=False):
    import concourse.bass as bass  # noqa: F401
    import concourse.mybir as mybir
    import concourse.tile as tile
    from concourse import bacc

    dt = mybir.dt
    f16, f32 = dt.float16, dt.float32
    AF = mybir.ActivationFunctionType
    AX = mybir.AxisListType

    TT = t // 128       # t tiles
    TC = t // 512       # t chunks (4 t-tiles each)
    SC = s // 512       # s chunks for scores matmul
    HT = h // 128       # h tiles (also k-steps over H)
    ST = s // 128       # s tiles (k-steps over S)
    DT = 2 * h // 128   # k-steps over 2H for the final matmul
    HC = h // 512       # h' chunks for the final matmul

    nc = bacc.Bacc()
    m16 = nc.declare_dram_parameter("m16", [s, bl, h], f16, isOutput=False)
    qT16 = nc.declare_dram_parameter("qT16", [bl, h, t], f16, isOutput=False)
    mT16 = nc.declare_dram_parameter("mT16", [bl, h, s], f16, isOutput=False)
    qlT16 = nc.declare_dram_parameter("qlT16", [bl, h, t], f16, isOutput=False)
    mlT16 = nc.declare_dram_parameter("mlT16", [bl, h, s], f16, isOutput=False)
    wt16 = nc.declare_dram_parameter("wt16", [2 * h, h], f16, isOutput=False)
    b16 = nc.declare_dram_parameter("b16", [1, h], f16, isOutput=False)
    out_e = nc.declare_dram_parameter("out", [t, bl, h], f32, isOutput=True)
    attn_e = nc.declare_dram_parameter("attn", [bl, t, s], f32, isOutput=True)

    with ExitStack() as ctx:
        tc = ctx.enter_context(tile.TileContext(nc))
        consts = ctx.enter_context(tc.tile_pool(name="consts", bufs=1))
        lay = ctx.enter_context(tc.tile_pool(name="lay", bufs=1))
        work1 = ctx.enter_context(tc.tile_pool(name="work1", bufs=1))
        ctxp = ctx.enter_context(tc.tile_pool(name="ctxp", bufs=2))
        e16p = ctx.enter_context(tc.tile_pool(name="e16p", bufs=2))
        stats = ctx.enter_context(tc.tile_pool(name="stats", bufs=4))
        # one unified psum pool: 4 rotating slots of [128,1024] (2 banks each)
        psq = ctx.enter_context(tc.tile_pool(name="psq", bufs=4, space="PSUM"))
        dramp = ctx.enter_context(tc.tile_pool(name="dramp", bufs=2,
                                               space="DRAM"))
        hwdge = (nc.sync, nc.scalar)  # spread engine-serial DMA transposes

        # W^T (2H,H) as DT tiles of [128, H]; loaded after the first batch's
        # hi layouts (WT is only needed late, by the out-matmuls)
        WT = consts.tile([128, DT, h], f16, tag="WT")
        bsb = consts.tile([1, h], f16, tag="bsb")
        nc.sync.dma_start(out=bsb[:], in_=b16[:])
        ones = consts.tile([1, 128], f16, tag="ones")
        nc.vector.memset(ones[:], 1.0)

        def emit_out(QTb, ctxTb, bb, tch, i):
            # out = tanh([ctx, Q] @ W^T + b) for t-tile i of chunk tch
            ti = tch * 4 + i
            osb = work1.tile([128, h], f32, tag="osb")
            po = psq.tile([128, 1024], f32, tag="psq", name=f"po_{bb}_{ti}")
            for hp in range(HC):
                pos = po[:, 512 * hp:512 * (hp + 1)]
                for kd in range(HT):
                    nc.tensor.matmul(
                        pos,
                        lhsT=ctxTb[:, kd, 128 * i:128 * (i + 1)],
                        rhs=WT[:, kd, 512 * hp:512 * (hp + 1)],
                        start=(kd == 0), stop=False)
                for kd in range(HT):
                    nc.tensor.matmul(
                        pos,
                        lhsT=QTb[kd][:, 128 * ti:128 * (ti + 1)],
                        rhs=WT[:, HT + kd, 512 * hp:512 * (hp + 1)],
                        start=False, stop=False)
                nc.tensor.matmul(
                    pos, lhsT=ones,
                    rhs=bsb[:, 512 * hp:512 * (hp + 1)],
                    start=False, stop=True)
                nc.scalar.activation(
                    out=osb[:, 512 * hp:512 * (hp + 1)], in_=pos,
                    func=AF.Tanh)
            nc.gpsimd.dma_start(
                out=out_e[128 * ti:128 * (ti + 1), bb, :], in_=osb)

        def emit_ctx_pair(Mnb, attnTb, ctxTb, bb, tch, hq):
            # one pair of context^T h-tiles into a rotating psum slot
            pc = psq.tile([128, 1024], f32, tag="psq",
                          name=f"pc_{bb}_{tch}_{hq}")
            for j in range(ST):
                for u in range(2):
                    hh = 2 * hq + u
                    nc.tensor.matmul(
                        pc[:, 512 * u:512 * (u + 1)],
                        lhsT=Mnb[j][:, 128 * hh:128 * (hh + 1)],
                        rhs=attnTb[j // (ST // 4)][:, j % (ST // 4), :],
                        start=(j == 0), stop=(j == ST - 1))
            for u in range(2):
                nc.scalar.copy(out=ctxTb[:, 2 * hq + u, :],
                               in_=pc[:, 512 * u:512 * (u + 1)])

        # ctx work of chunk c is deferred and drained (1 item per t-tile)
        # during chunk c+1's scores; out work of chunk c drains right after
        # chunk c+1's transpose-reads are issued, so PE stays busy while the
        # reads stream on the sync engine
        pcwork, powork = [], []
        for b in range(bl):
            # per-batch input layouts (hi + lo residual for exact-ish scores),
            # as per-h-chunk tiles so consumers start as soon as each chunk
            # lands.  hi layouts first: scores pass 1 only needs QT/MT.
            QT = [lay.tile([128, t], f16, tag=f"QT{hc}", name=f"QT{hc}_{b}") for hc in range(HT)]
            MT = [lay.tile([128, s], f16, tag=f"MT{hc}", name=f"MT{hc}_{b}") for hc in range(HT)]
            QTl = [lay.tile([128, t], f16, tag=f"QTl{hc}", name=f"QTl{hc}_{b}") for hc in range(HT)]
            MTl = [lay.tile([128, s], f16, tag=f"MTl{hc}", name=f"MTl{hc}_{b}") for hc in range(HT)]
            Mn = [lay.tile([128, h], f16, tag=f"Mn{st}", name=f"Mn{st}_{b}") for st in range(ST)]
            for hc in range(HT):
                hsl = slice(128 * hc, 128 * (hc + 1))
                nc.sync.dma_start(out=QT[hc][:], in_=qT16[b, hsl, :])
                nc.sync.dma_start(out=MT[hc][:], in_=mT16[b, hsl, :])
            for hc in range(HT):
                hsl = slice(128 * hc, 128 * (hc + 1))
                nc.sync.dma_start(out=QTl[hc][:], in_=qlT16[b, hsl, :])
                nc.sync.dma_start(out=MTl[hc][:], in_=mlT16[b, hsl, :])
            for st in range(ST):
                nc.sync.dma_start(
                    out=Mn[st][:], in_=m16[128 * st:128 * (st + 1), b, :])

            for tch in range(TC):
                attnT = [work1.tile([128, ST // 4, 512], f16,
                                    tag=f"attnT{k}", name=f"attnT{k}_{b}_{tch}")
                         for k in range(4)]
                e16d = dramp.tile([512, s], f16, tag="e16d",
                                  name=f"e16d_{b}_{tch}")
                for i in range(4):
                    ti = tch * 4 + i
                    tsl = slice(128 * ti, 128 * (ti + 1))
                    # scores for t-tile ti, in two s-halves, each a rotating
                    # psum slot: next tile's matmuls never wait on this
                    # tile's softmax drain.
                    psh = []
                    mx = stats.tile([128, 2], f32, tag="mx")
                    pairs = ((QT, MT), (QTl, MT), (QT, MTl))
                    sh = s // 2
                    for hf in range(2):
                        ph = psq.tile([128, sh], mybir.dt.float32,
                                      tag="psq", name=f"ps_{b}_{ti}_{hf}")
                        psh.append(ph)
                        for pi, (L, R) in enumerate(pairs):
                            for kh in range(HT):
                                for s2 in range(sh // 512):
                                    nc.tensor.matmul(
                                        ph[:, 512 * s2:512 * (s2 + 1)],
                                        lhsT=L[kh][:, tsl],
                                        rhs=R[kh][:, sh * hf + 512 * s2:
                                                   sh * hf + 512 * (s2 + 1)],
                                        start=(pi == 0 and kh == 0),
                                        stop=(pi == 2 and kh == HT - 1))
                        nc.vector.reduce_max(out=mx[:, hf:hf + 1], in_=ph[:],
                                             axis=AX.X)
                    negmax = stats.tile([128, 1], f32, tag="negmax")
                    nc.vector.reduce_max(out=negmax, in_=mx[:], axis=AX.X,
                                         negate=True)
                    e16 = e16p.tile([128, s], f16, tag="e16")
                    ssum = stats.tile([128, 2], f32, tag="ssum")
                    for hf in range(2):
                        nc.scalar.activation(
                            out=e16[:, sh * hf:sh * (hf + 1)],
                            in_=psh[hf][:], func=AF.Exp, bias=negmax,
                            scale=1.0, accum_out=ssum[:, hf:hf + 1])
                    rsum = stats.tile([128, 1], f32, tag="rsum")
                    nc.vector.reduce_sum(out=rsum, in_=ssum[:], axis=AX.X)
                    recip = stats.tile([128, 1], f32, tag="recip")
                    nc.vector.reciprocal(recip, rsum)
                    # normalize in place; attn output via casting SWDGE DMA,
                    # fp16 copy to DRAM scratch for the big transpose-reads
                    nc.vector.tensor_scalar_mul(e16, e16, recip)
                    nc.sync.dma_start(
                        out=e16d[128 * i:128 * (i + 1), :], in_=e16)
                    nc.gpsimd.dma_start(
                        out=attn_e[b, 128 * ti:128 * (ti + 1), :], in_=e16)
                    if b == 0 and tch == 0:
                        # spread weight loads; first needed by the outs of
                        # chunk 0, drained a whole chunk later
                        for kt in range(i * DT // 4, (i + 1) * DT // 4):
                            nc.sync.dma_start(
                                out=WT[:, kt, :],
                                in_=wt16[128 * kt:128 * (kt + 1), :])
                # attn^T for the whole chunk: ST transpose-reads of
                # [512,128] from the DRAM scratch, split over both HWDGE
                # engines (engine-serial, ~1.4us each)
                for j in range(ST):
                    nc.sync.dma_start_transpose(
                        attnT[j // (ST // 4)][:, j % (ST // 4), :],
                        e16d[:, 128 * j:128 * (j + 1)])
                # drain ALL of the previous chunk's ctx+out work here: its
                # reads finished a chunk ago, so this is dep-free PE filler
                # that covers this chunk's read/sem window
                while pcwork:
                    pcwork.pop(0)()
                while powork:
                    powork.pop(0)()

                ctxT = ctxp.tile([128, HT, 512], f16, tag="ctxT",
                                 name=f"ctxT_{b}_{tch}")
                for hq in range(HT // 2):
                    pcwork.append(
                        lambda Mnb=Mn, aT=attnT, cT=ctxT, bb=b, tc_=tch,
                        hq_=hq: emit_ctx_pair(Mnb, aT, cT, bb, tc_, hq_))
                for i in range(4):
                    powork.append(
                        lambda QTb=QT, cT=ctxT, bb=b, tc_=tch, i_=i:
                        emit_out(QTb, cT, bb, tc_, i_))
            # flush before the next batch's layouts overwrite QT/Mn (their
            # WAR would otherwise put deferred work behind the next batch's
            # scores in the PE stream -> deadlock)
            while pcwork:
                pcwork.pop(0)()
            while powork:
                powork.pop(0)()
    nc.finalize()
    return nc


def kernel(output, memory, attn_mask, W, b):
    from concourse.bass_utils import run_bass_kernel_spmd

    qf = np.asarray(output, dtype=np.float32).reshape(T, B, H)
    mf = np.asarray(memory, dtype=np.float32).reshape(S, B, H)
    q16 = qf.astype(np.float16)
    m16 = mf.astype(np.float16)
    q16l = (qf - q16.astype(np.float32)).astype(np.float16)
    m16l = (mf - m16.astype(np.float32)).astype(np.float16)
    # pre-transposed layouts (B, H, T/S): the kernel's matmul operand layout
    qT16 = np.ascontiguousarray(q16.transpose(1, 2, 0))
    mT16 = np.ascontiguousarray(m16.transpose(1, 2, 0))
    qlT16 = np.ascontiguousarray(q16l.transpose(1, 2, 0))
    mlT16 = np.ascontiguousarray(m16l.transpose(1, 2, 0))
    wt16 = np.ascontiguousarray(np.asarray(W, dtype=np.float32).T,
                                dtype=np.float16)          # (2H, H)
    b16 = np.asarray(b, dtype=np.float16).reshape(1, H)

    nc = build_nc()
    in_maps = []
    for c in range(NCORES):
        lo, hi = c * BL, (c + 1) * BL
        in_maps.append({
            "m16": np.ascontiguousarray(m16[:, lo:hi, :]),
            "qT16": np.ascontiguousarray(qT16[lo:hi]),
            "mT16": np.ascontiguousarray(mT16[lo:hi]),
            "qlT16": np.ascontiguousarray(qlT16[lo:hi]),
            "mlT16": np.ascontiguousarray(mlT16[lo:hi]),
            "wt16": wt16,
            "b16": b16,
        })
    res = run_bass_kernel_spmd(nc, in_maps, core_ids=list(range(NCORES)))
    out_full = np.concatenate([r["out"] for r in res.results], axis=1)
    attn_full = np.concatenate([r["attn"] for r in res.results], axis=0)
    return out_full, attn_full
